# revision 51
# baseline (speedup 1.0000x reference)
"""Trainium2 Bass kernel for nn_DimeNetOutput (gnn message passing).

Computes, for E=1M edges / N=100K nodes / D=64:
    x        = (edge_attr @ We + be) * msg_emb          # [E, 64]
    node_emb = segment_sum(x, edge_dst, N)              # [N, 64]
    h        = relu(node_emb @ W1 + b1)  (applied 3x, same weights)
    out      = h @ W4                                   # [N, 64]

Strategy (8 NeuronCores, node-range sharding -> no collectives):
  * Host: core c owns nodes [c*12500, (c+1)*12500).  Nodes are bucketed by
    column width W = ceil(deg/2) (quantized); a bucket is laid out as W
    contiguous "planes" of m columns each, so one SBUF column = 2 edges x
    64 features = 128 partitions, and a node's W columns sit at stride m.
    The segment sum is then W-1 pairwise plane adds on contiguous fp16.
  * Device, per 6144-col supertile: PE matmuls (band-packed attr, 32-row
    stationaries) -> (xlin+be)*msg via ACT Identity+bias then TT mult on
    DVE/Pool, or DVE scalar_tensor_tensor fused (GPSIMD cannot read PSUM,
    so only ACT/DVE consume matmul output) -> per-bucket plane-pair adds
    (multi-plane strided APs, one instr per level) leaving TWO partials
    per node (evens/odds) -> MLP layer 1 accumulates both partials in
    PSUM (absorbs the last tree level into PE), layers 2-4 chunked,
    relu on ACT/DVE -> fp16 output unpacked on host.
  * Work split across engines tuned against HW microbenches (DVE TT
    0.88ns/col, ACT conv 1.28, DVE STT 1.46, Pool TT 2.28 - the Q7 is
    ~2.3x slower than DVE; DMA ~346 GB/s/core, model-accurate).
"""

import os

import numpy as np

# ----------------------------------------------------------------- constants
E = 1_000_000
N = 100_000
A_DIM = 16
D = 64
NCORES = 8

NPC = N // NCORES          # 12500 nodes per core
STC = 6144                 # super-tile columns
BAND = 1024                # psum band columns (2 x 512)
NBAND = STC // BAND        # 4 bands per super-tile
MLP_CH = 500               # MLP chunk columns

# engine assignment knobs (tuned against CoreSim)
CONV_PATTERN = ["A", "A", "V"]                           # band convert engine
# A = ACT copy+bias then mult on DVE/Pool; V = DVE fused STT
# (GPSIMD/Pool cannot touch PSUM on TRN2 -- only ACT/DVE consume matmul out)
MULT_WEIGHTS = {"V": 0.55, "P": 0.45}   # x *= msg split (SBUF f16)
TREE_WEIGHTS = {"V": 0.75, "P": 0.25}   # tree add split (SBUF f16)
MLP_WEIGHTS = {"A": 0.65, "V": 0.35}    # relu split (PSUM src: ACT/DVE only)
WP_BUFS = 4                # supertile pipeline depth

_F16 = np.float16
_PROG = {}
_PLAN = {}


# ------------------------------------------------------------------ planning
def _quantize_w(w):
    """Allowed widths: exact 1..8, then even steps (keeps bucket count low)."""
    if w <= 8:
        return w
    for q in (10, 12, 14, 16, 20, 24, 32, 40, 48, 64):
        if w <= q:
            return q
    raise RuntimeError(f"node width {w} too large")


def build_plan(edge_dst):
    """Static layout shared by all cores (capacities = max over cores)."""
    dst = np.asarray(edge_dst).astype(np.int64).ravel()
    deg = np.bincount(dst, minlength=N).astype(np.int64)
    w_all = np.maximum(1, (deg + 1) // 2)
    wq_all = w_all.copy()
    hi = w_all > 8
    if hi.any():
        qs = np.array([10, 12, 14, 16, 20, 24, 32, 40, 48, 64], np.int64)
        wq_all[hi] = qs[np.searchsorted(qs, w_all[hi])]

    wq_pc = wq_all.reshape(NCORES, NPC)
    wset = np.unique(wq_all)
    # capacity per width = max count over cores, even
    caps = {}
    for w in wset:
        cnt = (wq_pc == w).sum(axis=1).max()
        caps[int(w)] = int(cnt + (cnt & 1))

    # blocks: pack the column space contiguously, splitting width-buckets at
    # super-tile boundaries (block = some nodes' planes, w*m cols, m even)
    blocks = []          # (w, m, colbase, nodebase)
    col = 0
    node = 0
    for w in sorted(caps):
        left = caps[w]
        while left > 0:
            room = STC - (col % STC)
            m = min(left, (room // w) & ~1)
            if m < 2:
                col += room          # gap-fill to next supertile
                continue
            blocks.append((w, m, col, node))
            col += w * m
            node += m
            left -= m
    ncols = -(-col // BAND) * BAND          # round up to a whole band
    nst = -(-ncols // STC)                  # last supertile may be short
    ncap = -(-node // 1000) * 1000
    return {
        "caps": caps, "blocks": blocks, "NCOLS": ncols, "NST": nst,
        "NCAP": ncap, "NREAL": node,
    }


# ------------------------------------------------------------- host packing
def pack_inputs(edge_attr, msg_emb, edge_dst, We, be, W1, b1, W4):
    """Build the 8 per-core input maps (numpy only); stores plan in _PLAN."""
    dst = np.asarray(edge_dst).astype(np.int64).ravel()
    attr = np.asarray(edge_attr, dtype=np.float32)
    msg = np.asarray(msg_emb, dtype=np.float32)
    We = np.asarray(We, dtype=np.float32)
    be = np.asarray(be, dtype=np.float32).ravel()
    W1 = np.asarray(W1, dtype=np.float32)
    b1 = np.asarray(b1, dtype=np.float32).ravel()
    W4 = np.asarray(W4, dtype=np.float32)
    assert dst.shape == (E,) and attr.shape == (E, A_DIM) and msg.shape == (E, D)

    plan = build_plan(dst)
    _PLAN.clear()
    _PLAN.update(plan)
    ncols, ncap = plan["NCOLS"], plan["NCAP"]

    deg = np.bincount(dst, minlength=N).astype(np.int64)
    w_all = np.maximum(1, (deg + 1) // 2)
    wq_all = w_all.copy()
    hi = w_all > 8
    if hi.any():
        qs = np.array([10, 12, 14, 16, 20, 24, 32, 40, 48, 64], np.int64)
        wq_all[hi] = qs[np.searchsorted(qs, w_all[hi])]

    order = np.argsort(dst, kind="stable")
    estart = np.zeros(N + 1, np.int64)
    np.cumsum(deg, out=estart[1:])

    # per-width block tables
    wlist = sorted(plan["caps"])
    widx = {w: i for i, w in enumerate(wlist)}
    blk_by_w = {w: [] for w in wlist}
    for (w, m, colbase, nodebase) in plan["blocks"]:
        blk_by_w[w].append((m, colbase, nodebase))

    # shared weights; we2 replicated into all 4 32-row bands (attr band pack)
    we2 = np.zeros((128, 128), _F16)
    for u in range(4):
        we2[32 * u:32 * u + 16, 0:64] = We
        we2[32 * u + 16:32 * u + 32, 64:128] = We
    w1s = np.concatenate([W1, W1], axis=0).astype(_F16)          # [128, 64]
    w1b = np.zeros((128, 128), _F16)
    w1b[0:64, 0:64] = W1
    w1b[64:128, 64:128] = W1
    w4b = np.zeros((128, 128), _F16)
    w4b[0:64, 0:64] = W4
    w4b[64:128, 64:128] = W4
    b1h = b1.reshape(64, 1).astype(np.float32)
    b1s = np.concatenate([b1, b1]).reshape(128, 1).astype(np.float32)
    bes = np.concatenate([be, be]).reshape(128, 1).astype(np.float32)

    in_maps = []
    perms = []
    for c in range(NCORES):
        nlo = c * NPC
        wq = wq_all[nlo:nlo + NPC]
        deg_c = deg[nlo:nlo + NPC]

        # node -> slot within its width bucket (stable order by node id)
        sort_idx = np.argsort(wq, kind="stable")
        slot_in_w = np.empty(NPC, np.int64)
        wc_sorted = wq[sort_idx]
        starts = np.searchsorted(wc_sorted, wlist, side="left")
        slot_all = np.arange(NPC)
        for i, w in enumerate(wlist):
            lo = starts[i]
            hi_ = starts[i + 1] if i + 1 < len(wlist) else np.searchsorted(
                wc_sorted, w, side="right")
            slot_in_w[sort_idx[lo:hi_]] = slot_all[lo:hi_] - lo

        # node -> (block colbase, block m, block nodebase)
        node_colbase = np.empty(NPC, np.int64)
        node_m = np.empty(NPC, np.int64)
        node_slotcol = np.empty(NPC, np.int64)   # i within block
        node_slot = np.empty(NPC, np.int64)      # global node slot
        for w in wlist:
            sel = wq == w
            if not sel.any():
                continue
            s = slot_in_w[sel]
            # walk blocks of this width
            bounds = np.cumsum([0] + [m for (m, _, _) in blk_by_w[w]])
            bi = np.searchsorted(bounds, s, side="right") - 1
            ms = np.array([m for (m, _, _) in blk_by_w[w]], np.int64)
            cbs = np.array([cb for (_, cb, _) in blk_by_w[w]], np.int64)
            nbs = np.array([nb for (_, _, nb) in blk_by_w[w]], np.int64)
            node_colbase[sel] = cbs[bi]
            node_m[sel] = ms[bi]
            node_slotcol[sel] = s - bounds[bi]
            node_slot[sel] = nbs[bi] + (s - bounds[bi])
        perms.append(node_slot.copy())

        # per-edge placement
        es = estart[nlo:nlo + NPC]
        eidx = order[es[0]:estart[nlo + NPC]]          # edges sorted by dst
        j = np.arange(len(eidx)) - np.repeat(es - es[0], deg_c)
        nd = np.repeat(np.arange(NPC), deg_c)
        plane = j // 2
        half = j % 2
        colpos = node_colbase[nd] + plane * node_m[nd] + node_slotcol[nd]

        a2 = np.zeros((32, ncols), np.float32)
        mT = np.zeros((128, ncols), np.float32)
        selA = half == 0
        selB = ~selA
        a2[0:16, colpos[selA]] = attr[eidx[selA]].T
        a2[16:32, colpos[selB]] = attr[eidx[selB]].T
        mT[0:64, colpos[selA]] = msg[eidx[selA]].T
        mT[64:128, colpos[selB]] = msg[eidx[selB]].T

        # band-pack attr: chunk q (512 cols) -> rows 32*(q%4), col 512*(q//4)
        apad = -(-ncols // 2048) * 2048
        if apad > ncols:
            a2 = np.concatenate(
                [a2, np.zeros((32, apad - ncols), np.float32)], axis=1)
        nq = apad // 512
        a2r = a2.reshape(32, nq, 512)
        ap_ = np.empty((128, apad // 4), np.float32)
        for u in range(4):
            ap_[32 * u:32 * (u + 1)] = a2r[:, u::4].reshape(32, apad // 4)

        in_maps.append({
            "attrp": ap_.astype(_F16), "msgT": mT.astype(_F16),
            "we2": we2, "w1s": w1s, "w1b": w1b, "w4b": w4b,
            "b1h": b1h, "b1s": b1s, "bes": bes,
        })
    _PLAN["perms"] = perms
    return in_maps


def unpack_output(results):
    """results: 8 dicts with 'outp' [128, NCAP//2] f16 -> [N, 64] f32."""
    ncap = _PLAN["NCAP"]
    perms = _PLAN["perms"]
    full = np.empty((N, D), np.float32)
    for c, r in enumerate(results):
        op_ = np.asarray(r["outp"], np.float32)          # [128, ncap//2]
        # node slot J: beta = (J % 1000) // 500, col = (J//1000)*500 + J%500
        J = perms[c]
        beta = (J % 1000) // 500
        colj = (J // 1000) * 500 + J % 500
        # gather [64] per node: rows 64*beta .. +64
        full[c * NPC:(c + 1) * NPC] = op_[
            (64 * beta)[:, None] + np.arange(64)[None, :], colj[:, None]]
    return full


# ---------------------------------------------------------- device program
class _RR:
    """Deterministic weighted round-robin engine picker."""

    def __init__(self, weights):
        self.w = dict(weights)
        self.credit = {k: 0.0 for k in weights}

    def pick(self):
        for k in self.credit:
            self.credit[k] += self.w[k]
        k = max(self.credit, key=lambda q: self.credit[q])
        self.credit[k] -= 1.0
        return k


def build_device_program(tc, outs, ins, rep=0):
    import concourse.mybir as mybir

    nc = tc.nc
    f16 = mybir.dt.float16
    f32 = mybir.dt.float32
    Alu = mybir.AluOpType
    Act = mybir.ActivationFunctionType

    blocks = _PLAN["blocks"]
    nst = _PLAN["NST"]
    ncap = _PLAN["NCAP"]
    nreal = _PLAN["NREAL"]
    outp = outs["outp"]

    # blocks by supertile
    st_blocks = [[] for _ in range(nst)]
    for (w, m, colbase, nodebase) in blocks:
        st_blocks[colbase // STC].append((w, m, colbase % STC, nodebase))

    mult_rr = _RR(MULT_WEIGHTS)
    tree_rr = _RR(TREE_WEIGHTS)
    mlp_rr = _RR(MLP_WEIGHTS)

    def eng(sel):
        return {"P": nc.gpsimd, "V": nc.vector}[sel]

    with tc.sbuf_pool(name=f"cpool{rep}", bufs=1) as cp:
        import concourse.mybir as _mb
        we2_t = cp.tile_from(ins["we2"], forced_dma_engine=_mb.EngineType.SP)
        w1s_t = cp.tile_from(ins["w1s"], forced_dma_engine=_mb.EngineType.SP)
        w1b_t = cp.tile_from(ins["w1b"], forced_dma_engine=_mb.EngineType.SP)
        w4b_t = cp.tile_from(ins["w4b"], forced_dma_engine=_mb.EngineType.SP)
        b1h_t = cp.tile_from(ins["b1h"], forced_dma_engine=_mb.EngineType.SP)
        b1s_t = cp.tile_from(ins["b1s"], forced_dma_engine=_mb.EngineType.SP)
        bes_t = cp.tile_from(ins["bes"], forced_dma_engine=_mb.EngineType.SP)
        # two partial sums per node: evens -> a, odds -> b; the MLP layer-1
        # matmul accumulates both (absorbs the last tree level into PE).
        n2a_t = cp.tile([128, ncap], f16)
        n2b_t = cp.tile([128, ncap], f16)
        # statically-zero regions of the b-partials: W<=2 blocks + pad tail
        zb = []
        for (w, m, colbase, nodebase) in blocks:
            if w <= 2:
                if zb and zb[-1][1] == nodebase:
                    zb[-1][1] = nodebase + m
                else:
                    zb.append([nodebase, nodebase + m])
        for lo, hi in zb:
            nc.gpsimd.memset(n2b_t[:, lo:hi], 0.0)
        if ncap > nreal:
            nc.gpsimd.memset(n2a_t[:, nreal:ncap], 0.0)
            nc.gpsimd.memset(n2b_t[:, nreal:ncap], 0.0)
        h1_t = cp.tile([128, ncap // 2], f16)
        h2_t = cp.tile([128, ncap // 2], f16)

        with tc.sbuf_pool(name=f"wpool{rep}", bufs=WP_BUFS) as wp, \
             tc.sbuf_pool(name=f"opool{rep}", bufs=3) as obp, \
             tc.tile_pool(name=f"pspool{rep}", bufs=3, space="PSUM") as pp, \
             tc.tile_pool(name=f"mpspool{rep}", bufs=2, space="PSUM") as mpp:

            # ---- MLP chunk emitters (called as node ranges complete) ----
            def mlp_conv(sel, out_ap, in_ap, bias_ap):
                if sel == "A":
                    nc.scalar.activation(out_ap, in_ap, Act.Relu, bias=bias_ap)
                else:
                    nc.vector.tensor_scalar(out_ap, in_ap, bias_ap, 0.0,
                                            op0=Alu.add, op1=Alu.max)

            def emit_l1(k):
                j0, beta = (k // 2) * MLP_CH, k % 2
                sl = slice(k * MLP_CH, (k + 1) * MLP_CH)
                pt = mpp.tile([64, MLP_CH], f32, tag="mp", name="pt1")
                nc.tensor.matmul(pt[:], w1s_t[:], n2a_t[:, sl],
                                 start=True, stop=False)
                nc.tensor.matmul(pt[:], w1s_t[:], n2b_t[:, sl],
                                 start=False, stop=True)
                mlp_conv(mlp_rr.pick(),
                         h1_t[64 * beta:64 * beta + 64, j0:j0 + MLP_CH],
                         pt[:], b1h_t[:, 0:1])

            def emit_l234(c0):
                w = MLP_CH
                pt = mpp.tile([128, w], f32, tag="mp", name="pt2")
                nc.tensor.matmul(pt[:], w1b_t[:], h1_t[:, c0:c0 + w],
                                 start=True, stop=True)
                mlp_conv(mlp_rr.pick(), h2_t[:, c0:c0 + w], pt[:],
                         b1s_t[:, 0:1])
                pt = mpp.tile([128, w], f32, tag="mp", name="pt3")
                nc.tensor.matmul(pt[:], w1b_t[:], h2_t[:, c0:c0 + w],
                                 start=True, stop=True)
                mlp_conv(mlp_rr.pick(), h1_t[:, c0:c0 + w], pt[:],
                         b1s_t[:, 0:1])
                pt = mpp.tile([128, w], f32, tag="mp", name="pt4")
                nc.tensor.matmul(pt[:], w4b_t[:], h1_t[:, c0:c0 + w],
                                 start=True, stop=True)
                ob = obp.tile([128, MLP_CH], f16, tag="ob", name="ob")
                sel = mlp_rr.pick()
                if sel == "A":
                    nc.scalar.activation(ob[:, 0:w], pt[:], Act.Identity)
                else:
                    nc.vector.tensor_copy(ob[:, 0:w], pt[:])
                nc.sync.dma_start(outp[:, c0:c0 + w], ob[:, 0:w])

            # ------------------------------- phase 1 + interleaved MLP ----
            done_nodes = 0     # node cols finalized (tree emitted)
            l1_done = 0        # L1 chunks emitted
            l234_done = 0

            def drain_mlp(limit):
                nonlocal l1_done, l234_done
                while (l1_done + 1) * MLP_CH <= limit:
                    emit_l1(l1_done)
                    l1_done += 1
                # layer1 chunk pair (2k, 2k+1) fills h1 cols [k*500,(k+1)*500)
                while (l234_done + 1) * 2 <= l1_done:
                    emit_l234(l234_done * MLP_CH)
                    l234_done += 1

            ncols = _PLAN["NCOLS"]

            def emit_front(st):
                """DMA + matmuls + convert + mult for one supertile."""
                st_cols = min(STC, ncols - st * STC)
                nb = st_cols // BAND
                nch = st_cols // 512
                acols = -(-nch // 4) * 512
                attr_t = wp.tile([128, STC // 4], f16, tag="attr")
                adma = nc.sync
                adma.dma_start(
                    attr_t[:, 0:acols],
                    ins["attrp"][:, st * (STC // 4):st * (STC // 4) + acols])
                msg_t = wp.tile([128, STC], f16, tag="msg")
                for b in range(nb):
                    nc.sync.dma_start(
                        msg_t[:, b * BAND:(b + 1) * BAND],
                        ins["msgT"][:, st * STC + b * BAND:
                                    st * STC + (b + 1) * BAND])
                x_t = wp.tile([128, STC], f16, tag="x")

                for b in range(nb):
                    c0 = b * BAND
                    ps_t = pp.tile([128, BAND], f32, tag="ps")
                    for qq in range(BAND // 512):
                        q = st * (STC // 512) + b * (BAND // 512) + qq
                        u = q % 4
                        bcol = 512 * (q // 4) - st * (STC // 4)
                        nc.tensor.matmul(
                            ps_t[:, 512 * qq:512 * (qq + 1)],
                            we2_t[32 * u:32 * (u + 1), :],
                            attr_t[32 * u:32 * (u + 1), bcol:bcol + 512],
                            start=True, stop=True,
                            tile_position=(32 * u, 0))
                    sel = CONV_PATTERN[(st * NBAND + b) % len(CONV_PATTERN)]
                    if sel == "V":
                        # fused (psum + be) * msg -> f16
                        nc.vector.scalar_tensor_tensor(
                            x_t[:, c0:c0 + BAND], ps_t[:], bes_t[:, 0:1],
                            msg_t[:, c0:c0 + BAND],
                            op0=Alu.add, op1=Alu.mult)
                    else:
                        nc.scalar.activation(x_t[:, c0:c0 + BAND],
                                             ps_t[:], Act.Identity,
                                             bias=bes_t[:, 0:1])
                        eng(mult_rr.pick()).tensor_tensor(
                            x_t[:, c0:c0 + BAND],
                            x_t[:, c0:c0 + BAND],
                            msg_t[:, c0:c0 + BAND], op=Alu.mult)
                return x_t

            def merge_planes(x_t, cb, m, planes, dest):
                """Pairwise-add plane set into dest ([128, m] AP)."""
                import concourse.ap as cap
                if len(planes) == 1:
                    nc.vector.tensor_copy(
                        dest, x_t[:, cb + planes[0] * m: cb + planes[0] * m + m])
                    return
                active = list(planes)
                while len(active) > 1:
                    pairs = list(zip(active[0::2], active[1::2]))
                    leftover = active[len(pairs) * 2:]
                    last = len(pairs) == 1 and not leftover
                    if len(pairs) == 1:
                        p0, p1 = pairs[0]
                        a_ap = x_t[:, cb + p0 * m: cb + p0 * m + m]
                        b_ap = x_t[:, cb + p1 * m: cb + p1 * m + m]
                        o_ap = dest if last else a_ap
                    else:
                        stride = (pairs[1][0] - pairs[0][0]) * m
                        k = len(pairs)

                        def v3(plane_off):
                            a = x_t[:, cb + plane_off * m: cb + plane_off * m + m]
                            l = list(a.ap)
                            return cap.AP(
                                tensor=a.tensor, offset=a.offset,
                                ap=[l[0], (stride, k), l[1]],
                                const_val=a.const_val,
                                runtime_checks=a.runtime_checks,
                                dep_tracking_offset=a.dep_tracking_offset)
                        a_ap = v3(pairs[0][0])
                        b_ap = v3(pairs[0][1])
                        o_ap = a_ap
                    eng(tree_rr.pick()).tensor_tensor(
                        o_ap, a_ap, b_ap, op=Alu.add)
                    active = [p for p, _ in pairs] + leftover

            def emit_back(st, x_t):
                """Per-block trees -> two partials per node."""
                nonlocal done_nodes
                for (w, m, cb, nodebase) in st_blocks[st]:
                    da = n2a_t[:, nodebase:nodebase + m]
                    db = n2b_t[:, nodebase:nodebase + m]
                    if w <= 2:
                        # full merge into a; b statically zero
                        merge_planes(x_t, cb, m, list(range(w)), da)
                    else:
                        merge_planes(x_t, cb, m, list(range(0, w, 2)), da)
                        merge_planes(x_t, cb, m, list(range(1, w, 2)), db)
                    done_nodes = max(done_nodes, nodebase + m)

            # software pipeline: trees/MLP for supertile st are emitted after
            # supertile st+LAG's front half, so no engine's in-order stream
            # stalls on a not-yet-computed dependency.
            LAG = 2
            stash = {}
            for st in range(nst + LAG):
                if st < nst:
                    stash[st] = emit_front(st)
                if st >= LAG:
                    emit_back(st - LAG, stash.pop(st - LAG))
                    drain_mlp(done_nodes)

            # tail: pad nodes (zero) + remaining chunks
            drain_mlp(ncap)


def build_program(repeat=1):
    """Build (once) the Bacc program + dram tensor APs (uses _PLAN)."""
    key = (_PLAN["NCOLS"], _PLAN["NCAP"], tuple(_PLAN["blocks"]), repeat)
    if _PROG.get("key") == key:
        return _PROG["nc"]
    import concourse.bacc as bacc
    import concourse.mybir as mybir
    import concourse.tile as tile

    nc = bacc.Bacc("TRN2", debug=False, enable_asserts=False)
    f16, f32 = mybir.dt.float16, mybir.dt.float32
    ncols, ncap = _PLAN["NCOLS"], _PLAN["NCAP"]
    ins = {
        "attrp": nc.dram_tensor("attrp", [128, (-(-ncols // 2048) * 2048) // 4],
                                f16, kind="ExternalInput").ap(),
        "msgT": nc.dram_tensor("msgT", [128, ncols], f16,
                               kind="ExternalInput").ap(),
        "we2": nc.dram_tensor("we2", [128, 128], f16, kind="ExternalInput").ap(),
        "w1s": nc.dram_tensor("w1s", [128, 64], f16, kind="ExternalInput").ap(),
        "w1b": nc.dram_tensor("w1b", [128, 128], f16, kind="ExternalInput").ap(),
        "w4b": nc.dram_tensor("w4b", [128, 128], f16, kind="ExternalInput").ap(),
        "b1h": nc.dram_tensor("b1h", [64, 1], f32, kind="ExternalInput").ap(),
        "b1s": nc.dram_tensor("b1s", [128, 1], f32, kind="ExternalInput").ap(),
        "bes": nc.dram_tensor("bes", [128, 1], f32, kind="ExternalInput").ap(),
    }
    outs = {
        "outp": nc.dram_tensor("outp", [128, ncap // 2], f16,
                               kind="ExternalOutput").ap(),
    }
    with tile.TileContext(nc) as tc:
        for rep in range(repeat):
            build_device_program(tc, outs, ins, rep=rep)
    nc.compile()
    _PROG["nc"] = nc
    _PROG["key"] = key
    return nc


# ------------------------------------------------------------------ kernel
def kernel(edge_attr, msg_emb, edge_dst, num_nodes, We, be, W1, b1, W4,
           **_unused):
    assert int(num_nodes) == N
    in_maps = pack_inputs(edge_attr, msg_emb, edge_dst, We, be, W1, b1, W4)
    nc = build_program()

    from concourse.bass_utils import run_bass_kernel_spmd
    trace = os.environ.get("GNN_TRACE", "0") == "1"
    res = run_bass_kernel_spmd(nc, in_maps, core_ids=list(range(NCORES)),
                               trace=trace)
    kernel.last_results = res
    return unpack_output(res.results)
